# revision 1
# baseline (speedup 1.0000x reference)
"""3x3 conv (256->256, stride 1, pad 1) as implicit GEMM on 8 TRN2 NeuronCores.

Data-parallel over batch: 32 images -> 4 per core; weight/bias replicated.

Per core, per image: x is resident in SBUF as two [128, 56, 58] channel
tiles (zero columns at w=0 and w=57 provide the horizontal conv padding;
fp32r matmuls require even innermost counts and 8B-aligned PSUM offsets,
so column-clipped taps are illegal — the pad columns keep every tap a
full 56-wide window). For each output row-tile of 8 rows and each of 2 output-channel
tiles, 18 matmuls (9 conv taps x 2 input-channel tiles) accumulate into a
PSUM tile [128, 8, 56] using float32r (1 cycle/row on the PE vs 4 for
plain fp32). Padding is handled by clipping each tap's row/col range via
3D access patterns; the center tap runs first with start=True so every
PSUM element's first write is an overwrite. Bias is fused into the
PSUM->SBUF eviction on the scalar engine.

The host wrapper pre-reorders the weight to [c, tap, ci, co, o] so lhsT
tiles ([c_in 128, c_out 128] per tap) DMA straight into SBUF.
"""

from contextlib import ExitStack

import numpy as np

import os

import concourse.bass as bass  # noqa: F401  (bass types used via tc/nc)
import concourse.tile as tile
from concourse import bacc, mybir
from concourse.bass_utils import run_bass_kernel_spmd

N_CORES = 8
N_TOTAL = 32
N_PER = N_TOTAL // N_CORES  # 4 images per core
C = 256
H = W = 56
RT = 8          # output rows per PSUM tile -> 8*56 = 448 <= 512 (one bank)
NRT = H // RT   # 7 row tiles
F32 = mybir.dt.float32
F32R = mybir.dt.float32r
# compute dtype for the matmul operands (storage + PE streaming format)
_MM_DT_NAME = os.environ.get("CONV_MM_DTYPE", "float32r")
MM_DT = getattr(mybir.dt, _MM_DT_NAME)
MM_NP = mybir.dt.np(MM_DT)

_CACHE = {}


def _build():
    nc = bacc.Bacc(
        "TRN2", target_bir_lowering=False, debug=False, num_devices=N_CORES
    )
    # xs/wt are declared float32r: same bits as fp32, but the PE reads the
    # matmul operands in its fast relaxed-precision mode (1 cycle/row vs 4
    # for exact fp32; measured rel err ~1.6e-4 at K=128).
    xs = nc.dram_tensor(
        "xs", [N_PER, C, H, W], MM_DT, kind="ExternalInput"
    ).ap()
    wt = nc.dram_tensor(
        "wt", [128, 9, 2, 2, 128], MM_DT, kind="ExternalInput"
    ).ap()
    b2 = nc.dram_tensor("b2", [128, 2], F32, kind="ExternalInput").ap()
    out = nc.dram_tensor(
        "out", [N_PER, C, H, W], F32, kind="ExternalOutput"
    ).ap()

    # Accumulation order: center tap (full coverage) first so its
    # start=True write touches every element of the PSUM tile.
    order = [(1, 1, 0)]
    for ci in (0, 1):
        for kh in range(3):
            for kw in range(3):
                if (kh, kw, ci) != (1, 1, 0):
                    order.append((kh, kw, ci))

    with tile.TileContext(nc) as tc, ExitStack() as ctx:
        wpool = ctx.enter_context(tc.tile_pool(name="w", bufs=1))
        spool = ctx.enter_context(tc.tile_pool(name="s", bufs=2))
        bpool = ctx.enter_context(tc.tile_pool(name="b", bufs=1))
        xpool = ctx.enter_context(tc.tile_pool(name="x", bufs=1))
        opool = ctx.enter_context(tc.tile_pool(name="o", bufs=4))
        ppool = ctx.enter_context(tc.tile_pool(name="p", bufs=4, space="PSUM"))

        # w/b loads go on the scalar engine's DMA ring so they overlap
        # with the first x loads on the sync ring.
        w_sb = wpool.tile([128, 9, 2, 2, 128], MM_DT)
        nc.scalar.dma_start(w_sb[:], wt[:, :, :, :, :])
        b_sb = bpool.tile([128, 2], F32)
        nc.scalar.dma_start(b_sb[:], b2[:, :])

        # Pad-column zeroing: memset cannot write float32r, but a DVE
        # tensor_copy f32 -> f32r can (it performs the f32r rounding).
        z_sb = bpool.tile([128, H, 1], F32)
        nc.vector.memset(z_sb[:], 0.0)

        # persistent, manually double-buffered x tiles: [slot][ci]
        x_tiles = []
        for slot in range(2):
            row = []
            for ci in range(2):
                t = xpool.tile(
                    [128, H, W + 2], MM_DT, tag=f"x{slot}{ci}"
                )
                nc.vector.tensor_copy(t[:, :, 0:1], z_sb[:])
                nc.vector.tensor_copy(t[:, :, W + 1 : W + 2], z_sb[:])
                row.append(t)
            x_tiles.append(row)

        for n in range(N_PER):
            x_sb = x_tiles[n % 2]
            for ci in range(2):
                # contiguous load to staging (fast DMA), then VectorE
                # restrides into the padded tile (DVE is otherwise idle)
                stg = spool.tile([128, H, W], MM_DT, tag=f"s{ci}")
                eng = nc.sync if ci == 0 else nc.scalar
                eng.dma_start(
                    stg[:], xs[n, ci * 128 : (ci + 1) * 128, :, :]
                )
                nc.vector.tensor_copy(
                    x_sb[ci][:, :, 1 : W + 1], stg[:]
                )
            for rt in range(NRT):
                h0 = rt * RT
                for co in range(2):
                    ps = ppool.tile([128, RT, W], F32)
                    for i, (kh, kw, ci) in enumerate(order):
                        dh = kh - 1
                        r0 = max(h0, -dh)
                        r1 = min(h0 + RT, H - dh)
                        lhsT = w_sb[:, kh * 3 + kw, ci, co, :]
                        rhs = x_sb[ci][:, r0 + dh : r1 + dh, kw : kw + W]
                        nc.tensor.matmul(
                            ps[:, r0 - h0 : r1 - h0, :],
                            lhsT,
                            rhs,
                            start=(i == 0),
                            stop=(i == len(order) - 1),
                        )
                    o_sb = opool.tile([128, RT, W], F32)
                    nc.scalar.activation(
                        o_sb[:],
                        ps[:],
                        mybir.ActivationFunctionType.Identity,
                        bias=b_sb[:, co : co + 1],
                    )
                    nc.scalar.dma_start(
                        out[n, co * 128 : (co + 1) * 128, h0 : h0 + RT, :],
                        o_sb[:],
                    )
    nc.compile()
    return nc


def _get_nc():
    if "nc" not in _CACHE:
        _CACHE["nc"] = _build()
    return _CACHE["nc"]


def _in_maps(x, weight, bias):
    x = np.ascontiguousarray(np.asarray(x, dtype=np.float32).astype(MM_NP))
    weight = np.asarray(weight, dtype=np.float32)
    bias = np.asarray(bias, dtype=np.float32)
    # weight[co*128+o, (ci*128+c)*9 + (kh*3+kw)] -> wt[c, tap, ci, co, o]
    wt = np.ascontiguousarray(
        weight.reshape(2, 128, 2, 128, 9).transpose(3, 4, 2, 0, 1).astype(MM_NP)
    )
    b2 = np.ascontiguousarray(bias.reshape(2, 128).T)
    return [
        {"xs": x[i * N_PER : (i + 1) * N_PER], "wt": wt, "b2": b2}
        for i in range(N_CORES)
    ]


def _run(x, weight, bias, trace=False):
    res = run_bass_kernel_spmd(
        _get_nc(),
        _in_maps(x, weight, bias),
        core_ids=list(range(N_CORES)),
        trace=trace,
    )
    out = np.concatenate(
        [res.results[i]["out"] for i in range(N_CORES)], axis=0
    )
    return out, res


def kernel(x, weight, bias):
    out, _ = _run(x, weight, bias, trace=False)
    return out


def run_profiled(x, weight, bias):
    out, res = _run(x, weight, bias, trace=True)
    return out, res.exec_time_ns



# revision 2
# speedup vs baseline: 1.1268x; 1.1268x over previous
"""3x3 conv (256->256, stride 1, pad 1) as implicit GEMM on 8 TRN2 NeuronCores.

Data-parallel over batch: 32 images -> 4 per core; weight/bias replicated.

Per core, per image: x is resident in SBUF as two [128, 56, 58] channel
tiles (zero columns at w=0 and w=57 provide the horizontal conv padding,
keeping every tap a full 56-wide window). For each output row-tile of 8
rows and each of 2 output-channel tiles, 18 matmuls (9 conv taps x 2
input-channel tiles) accumulate into a PSUM tile [128, 8, 56]. Operands
are bf16 (1 cycle/row on the PE, same as fp32r, but LDWEIGHTS is 2x
faster via fast-weight-load so the stationary reload fully hides under
the 187ns stream; fp32r's 187ns weight load was the critical path at
~210ns/matmul). Padding is handled by clipping each tap's row range via
3D access patterns; the center tap runs first with start=True so every
PSUM element's first write is an overwrite. Bias is fused into the
PSUM->SBUF eviction on the scalar engine.

DMA ring assignment spreads traffic: weights/bias on the gpsimd ring,
x staging loads on sync (ci=0) / scalar (ci=1), output stores alternate
sync/gpsimd. All 4 images have persistent SBUF tiles so the loads
prefetch ahead of compute.

The host wrapper pre-reorders the weight to [c, tap, ci, co, o] so lhsT
tiles ([c_in 128, c_out 128] per tap) DMA straight into SBUF.
"""

from contextlib import ExitStack

import numpy as np

import os

import concourse.bass as bass  # noqa: F401  (bass types used via tc/nc)
import concourse.tile as tile
from concourse import bacc, mybir
from concourse.bass_utils import run_bass_kernel_spmd

N_CORES = 8
N_TOTAL = 32
N_PER = N_TOTAL // N_CORES  # 4 images per core
C = 256
H = W = 56
RT = 8          # output rows per PSUM tile -> 8*56 = 448 <= 512 (one bank)
NRT = H // RT   # 7 row tiles
F32 = mybir.dt.float32
# compute dtypes for the matmul operands (storage + PE streaming format)
_X_DT_NAME = os.environ.get("CONV_X_DTYPE", "bfloat16")
_W_DT_NAME = os.environ.get("CONV_W_DTYPE", "bfloat16")
X_DT = getattr(mybir.dt, _X_DT_NAME)
W_DT = getattr(mybir.dt, _W_DT_NAME)
X_NP = mybir.dt.np(X_DT)
W_NP = mybir.dt.np(W_DT)

_CACHE = {}


def _build():
    nc = bacc.Bacc(
        "TRN2", target_bir_lowering=False, debug=False, num_devices=N_CORES
    )
    xs = nc.dram_tensor(
        "xs", [N_PER, C, H, W], X_DT, kind="ExternalInput"
    ).ap()
    wt = nc.dram_tensor(
        "wt", [128, 9, 2, 2, 128], W_DT, kind="ExternalInput"
    ).ap()
    b2 = nc.dram_tensor("b2", [128, 2], F32, kind="ExternalInput").ap()
    out = nc.dram_tensor(
        "out", [N_PER, C, H, W], F32, kind="ExternalOutput"
    ).ap()

    # Accumulation order: center tap (full coverage) first so its
    # start=True write touches every element of the PSUM tile.
    order = [(1, 1, 0)]
    for ci in (0, 1):
        for kh in range(3):
            for kw in range(3):
                if (kh, kw, ci) != (1, 1, 0):
                    order.append((kh, kw, ci))

    with tile.TileContext(nc) as tc, ExitStack() as ctx:
        wpool = ctx.enter_context(tc.tile_pool(name="w", bufs=1))
        spool = ctx.enter_context(tc.tile_pool(name="s", bufs=2))
        bpool = ctx.enter_context(tc.tile_pool(name="b", bufs=1))
        xpool = ctx.enter_context(tc.tile_pool(name="x", bufs=1))
        opool = ctx.enter_context(tc.tile_pool(name="o", bufs=4))
        ppool = ctx.enter_context(tc.tile_pool(name="p", bufs=4, space="PSUM"))

        # w/b loads go on the gpsimd DMA ring so they overlap with the
        # first x loads on the sync/scalar rings.
        w_sb = wpool.tile([128, 9, 2, 2, 128], W_DT)
        nc.gpsimd.dma_start(w_sb[:], wt[:, :, :, :, :])
        b_sb = bpool.tile([128, 2], F32)
        nc.gpsimd.dma_start(b_sb[:], b2[:, :])

        # Pad-column zeroing: DVE tensor_copy f32 -> X_DT performs the
        # dtype conversion (memset can't write all dtypes).
        z_sb = bpool.tile([128, H, 1], F32)
        nc.vector.memset(z_sb[:], 0.0)

        # persistent x tiles for all 4 images: [n][ci]
        x_tiles = []
        for n in range(N_PER):
            row = []
            for ci in range(2):
                t = xpool.tile(
                    [128, H, W + 2], X_DT, tag=f"x{n}{ci}"
                )
                nc.vector.tensor_copy(t[:, :, 0:1], z_sb[:])
                nc.vector.tensor_copy(t[:, :, W + 1 : W + 2], z_sb[:])
                row.append(t)
            x_tiles.append(row)

        for n in range(N_PER):
            x_sb = x_tiles[n]
            for ci in range(2):
                # contiguous load to staging (fast DMA), then VectorE
                # restrides into the padded tile (DVE is otherwise idle)
                stg = spool.tile([128, H, W], X_DT, tag=f"s{ci}")
                eng = nc.sync if ci == 0 else nc.scalar
                eng.dma_start(
                    stg[:], xs[n, ci * 128 : (ci + 1) * 128, :, :]
                )
                nc.vector.tensor_copy(
                    x_sb[ci][:, :, 1 : W + 1], stg[:]
                )
            for rt in range(NRT):
                h0 = rt * RT
                for co in range(2):
                    ps = ppool.tile([128, RT, W], F32)
                    for i, (kh, kw, ci) in enumerate(order):
                        dh = kh - 1
                        r0 = max(h0, -dh)
                        r1 = min(h0 + RT, H - dh)
                        lhsT = w_sb[:, kh * 3 + kw, ci, co, :]
                        rhs = x_sb[ci][:, r0 + dh : r1 + dh, kw : kw + W]
                        nc.tensor.matmul(
                            ps[:, r0 - h0 : r1 - h0, :],
                            lhsT,
                            rhs,
                            start=(i == 0),
                            stop=(i == len(order) - 1),
                        )
                    o_sb = opool.tile([128, RT, W], F32)
                    nc.scalar.activation(
                        o_sb[:],
                        ps[:],
                        mybir.ActivationFunctionType.Identity,
                        bias=b_sb[:, co : co + 1],
                    )
                    oeng = nc.sync if (rt + co) % 2 == 0 else nc.gpsimd
                    oeng.dma_start(
                        out[n, co * 128 : (co + 1) * 128, h0 : h0 + RT, :],
                        o_sb[:],
                    )
    nc.compile()
    return nc


def _get_nc():
    if "nc" not in _CACHE:
        _CACHE["nc"] = _build()
    return _CACHE["nc"]


def _in_maps(x, weight, bias):
    x = np.ascontiguousarray(np.asarray(x, dtype=np.float32).astype(X_NP))
    weight = np.asarray(weight, dtype=np.float32)
    bias = np.asarray(bias, dtype=np.float32)
    # weight[co*128+o, (ci*128+c)*9 + (kh*3+kw)] -> wt[c, tap, ci, co, o]
    wt = np.ascontiguousarray(
        weight.reshape(2, 128, 2, 128, 9).transpose(3, 4, 2, 0, 1).astype(W_NP)
    )
    b2 = np.ascontiguousarray(bias.reshape(2, 128).T)
    return [
        {"xs": x[i * N_PER : (i + 1) * N_PER], "wt": wt, "b2": b2}
        for i in range(N_CORES)
    ]


def _run(x, weight, bias, trace=False):
    res = run_bass_kernel_spmd(
        _get_nc(),
        _in_maps(x, weight, bias),
        core_ids=list(range(N_CORES)),
        trace=trace,
    )
    out = np.concatenate(
        [res.results[i]["out"] for i in range(N_CORES)], axis=0
    )
    return out, res


def kernel(x, weight, bias):
    out, _ = _run(x, weight, bias, trace=False)
    return out


def run_profiled(x, weight, bias):
    out, res = _run(x, weight, bias, trace=True)
    return out, res.exec_time_ns


# revision 7
# speedup vs baseline: 1.1689x; 1.0374x over previous
"""3x3 conv (256->256, stride 1, pad 1) as implicit GEMM on 8 TRN2 NeuronCores.

Data-parallel over batch: 32 images -> 4 per core; weight/bias replicated.

Per core, per image: x is resident in SBUF as two [128, 56, 58] channel
tiles (zero columns at w=0 and w=57 provide the horizontal conv padding,
keeping every tap a full 56-wide window). For each output row-tile of 8
rows and each of 2 output-channel tiles, 18 matmuls (9 conv taps x 2
input-channel tiles) accumulate into a PSUM tile [128, 8, 56]. Operands
are bf16 (1 cycle/row on the PE, same as fp32r, but LDWEIGHTS is 2x
faster via fast-weight-load so the stationary reload fully hides under
the ~187ns stream; fp32r's 187ns weight load was the critical path at
~210ns/matmul). Padding is handled by clipping each tap's row range via
3D access patterns; the center tap runs first with start=True so every
PSUM element's first write is an overwrite. Bias is fused into the
PSUM->SBUF eviction on the scalar engine (bf16 out, upcast on host).

Startup is choreographed so the PE starts ~8.5us in: the weight is
hosted in [c, co, tap', ci, o] order with the center tap first (tap'
order 4,0,1,2,3,5,6,7,8) and DMA'd in 3 pieces per co half on two
otherwise-idle rings (gpsimd: co0, vector: co1); x stages in 3 row
chunks so the first row-tile's data lands early; per-slice shadow-memory
dependency tracking lets the matmuls chase the arriving pieces. A few
dummy matmuls on zeros warm the PE clock gate (HAM) during the DMA wait
so the real matmuls run at 2.4 GHz from the start.
"""

from contextlib import ExitStack

import numpy as np

import os

import concourse.bass as bass  # noqa: F401  (bass types used via tc/nc)
import concourse.tile as tile
from concourse import bacc, mybir
from concourse.bass_utils import run_bass_kernel_spmd

N_CORES = 8
N_TOTAL = 32
N_PER = N_TOTAL // N_CORES  # 4 images per core
C = 256
H = W = 56
RT = 8          # output rows per PSUM tile -> 8*56 = 448 <= 512 (one bank)
NRT = H // RT   # 7 row tiles
F32 = mybir.dt.float32
# compute dtypes for the matmul operands (storage + PE streaming format)
_X_DT_NAME = os.environ.get("CONV_X_DTYPE", "bfloat16")
_W_DT_NAME = os.environ.get("CONV_W_DTYPE", "bfloat16")
X_DT = getattr(mybir.dt, _X_DT_NAME)
W_DT = getattr(mybir.dt, _W_DT_NAME)
X_NP = mybir.dt.np(X_DT)
W_NP = mybir.dt.np(W_DT)
N_DUMMY = int(os.environ.get("CONV_N_DUMMY", "6"))

# tap order in the hosted weight: center tap first so the first DMA piece
# carries the weights the first (start=True) matmuls need
WORDER = [4, 0, 1, 2, 3, 5, 6, 7, 8]
TIDX = {t: i for i, t in enumerate(WORDER)}
# x staging row chunks: row-tile rt reads rows 8rt-1..8rt+8, so chunk
# boundaries at 9/33 cover rt0 | rt1-3 | rt4-6 cumulatively
XCHUNKS = [(0, 9), (9, 33), (33, 56)]

_CACHE = {}


def _build():
    nc = bacc.Bacc(
        "TRN2", target_bir_lowering=False, debug=False, num_devices=N_CORES
    )
    xs = nc.dram_tensor(
        "xs", [N_PER, C, H, W], X_DT, kind="ExternalInput"
    ).ap()
    wt = nc.dram_tensor(
        "wt", [128, 2, 9, 2, 128], W_DT, kind="ExternalInput"
    ).ap()
    b2 = nc.dram_tensor("b2", [128, 2], F32, kind="ExternalInput").ap()
    out = nc.dram_tensor(
        "out", [N_PER, C, H, W], X_DT, kind="ExternalOutput"
    ).ap()

    # Accumulation order: center tap (full coverage) first so its
    # start=True write touches every element of the PSUM tile; then taps
    # in WORDER sequence (= weight DMA arrival order), both ci each.
    order = [(1, 1, 0), (1, 1, 1)]
    for t in WORDER[1:]:
        for ci in (0, 1):
            order.append((t // 3, t % 3, ci))

    with tile.TileContext(nc) as tc, ExitStack() as ctx:
        wpool = ctx.enter_context(tc.tile_pool(name="w", bufs=1))
        spool = ctx.enter_context(tc.tile_pool(name="s", bufs=2))
        bpool = ctx.enter_context(tc.tile_pool(name="b", bufs=1))
        xpool = ctx.enter_context(tc.tile_pool(name="x", bufs=1))
        opool = ctx.enter_context(tc.tile_pool(name="o", bufs=4))
        ppool = ctx.enter_context(tc.tile_pool(name="p", bufs=4, space="PSUM"))
        dpool = ctx.enter_context(tc.tile_pool(name="d", bufs=1, space="PSUM"))

        # PE warmup: a zero tile (memset early on the gpsimd queue) feeds
        # a few dummy matmuls into a scratch PSUM bank so the HAM clock
        # gate opens to 2.4 GHz while the real weight/x DMAs land.
        d_sb = bpool.tile([128, RT * W], X_DT)
        nc.gpsimd.memset(d_sb[:], 0.0)

        # weight DMA in tap pieces (center | next 4 | last 4) per co half
        # so the matmuls can start as soon as the early taps arrive. The
        # two center pieces ride at the head of the sync/scalar rings (in
        # front of the x chunks they unblock); the rest go on gpsimd.
        # Only sync/scalar/gpsimd rings can issue DMAs.
        w_sb = wpool.tile([128, 2, 9, 2, 128], W_DT)
        b_sb = bpool.tile([128, 2], F32)
        nc.gpsimd.dma_start(b_sb[:], b2[:, :])
        nc.sync.dma_start(w_sb[:, 0, 0:1], wt[:, 0, 0:1])
        nc.scalar.dma_start(w_sb[:, 1, 0:1], wt[:, 1, 0:1])
        for co, t0, t1 in ((0, 1, 5), (0, 5, 9), (1, 1, 5), (1, 5, 9)):
            nc.gpsimd.dma_start(w_sb[:, co, t0:t1], wt[:, co, t0:t1])

        # Pad-column zeroing: DVE tensor_copy f32 -> X_DT performs the
        # dtype conversion (memset can't write all dtypes).
        z_sb = bpool.tile([128, H, 1], F32)
        nc.vector.memset(z_sb[:], 0.0)

        ds = dpool.tile([128, RT * W], F32)
        for _ in range(N_DUMMY):
            nc.tensor.matmul(
                ds[:], d_sb[:, 0:128], d_sb[:], start=True, stop=True
            )

        # persistent x tiles for all 4 images: [n][ci]
        x_tiles = []
        for n in range(N_PER):
            row = []
            for ci in range(2):
                t = xpool.tile([128, H, W + 2], X_DT, tag=f"x{n}{ci}")
                row.append(t)
            x_tiles.append(row)

        for n in range(N_PER):
            x_sb = x_tiles[n]
            for ci in range(2):
                # pad columns for this image's tiles
                nc.vector.tensor_copy(x_sb[ci][:, :, 0:1], z_sb[:])
                nc.vector.tensor_copy(
                    x_sb[ci][:, :, W + 1 : W + 2], z_sb[:]
                )
                # contiguous load to staging in row chunks (fast DMA),
                # then VectorE restrides into the padded tile
                stg = spool.tile([128, H, W], X_DT, tag=f"s{ci}")
                eng = nc.sync if ci == 0 else nc.scalar
                for r0, r1 in XCHUNKS:
                    eng.dma_start(
                        stg[:, r0:r1],
                        xs[n, ci * 128 : (ci + 1) * 128, r0:r1, :],
                    )
                    nc.vector.tensor_copy(
                        x_sb[ci][:, r0:r1, 1 : W + 1], stg[:, r0:r1]
                    )
            for rt in range(NRT):
                h0 = rt * RT
                for co in range(2):
                    ps = ppool.tile([128, RT, W], F32)
                    for i, (kh, kw, ci) in enumerate(order):
                        dh = kh - 1
                        r0 = max(h0, -dh)
                        r1 = min(h0 + RT, H - dh)
                        lhsT = w_sb[:, co, TIDX[kh * 3 + kw], ci, :]
                        rhs = x_sb[ci][:, r0 + dh : r1 + dh, kw : kw + W]
                        nc.tensor.matmul(
                            ps[:, r0 - h0 : r1 - h0, :],
                            lhsT,
                            rhs,
                            start=(i == 0),
                            stop=(i == len(order) - 1),
                        )
                    o_sb = opool.tile([128, RT, W], X_DT)
                    nc.scalar.activation(
                        o_sb[:],
                        ps[:],
                        mybir.ActivationFunctionType.Identity,
                        bias=b_sb[:, co : co + 1],
                    )
                    oeng = nc.sync if (rt + co) % 2 == 0 else nc.gpsimd
                    oeng.dma_start(
                        out[n, co * 128 : (co + 1) * 128, h0 : h0 + RT, :],
                        o_sb[:],
                    )
    nc.compile()
    return nc


def _get_nc():
    if "nc" not in _CACHE:
        _CACHE["nc"] = _build()
    return _CACHE["nc"]


def _in_maps(x, weight, bias):
    x = np.ascontiguousarray(np.asarray(x, dtype=np.float32).astype(X_NP))
    weight = np.asarray(weight, dtype=np.float32)
    bias = np.asarray(bias, dtype=np.float32)
    # weight[co*128+o, (ci*128+c)*9 + (kh*3+kw)] -> wt[c, co, tap', ci, o]
    wt = weight.reshape(2, 128, 2, 128, 9).transpose(3, 0, 4, 2, 1)
    wt = np.ascontiguousarray(wt[:, :, WORDER].astype(W_NP))
    b2 = np.ascontiguousarray(bias.reshape(2, 128).T)
    return [
        {"xs": x[i * N_PER : (i + 1) * N_PER], "wt": wt, "b2": b2}
        for i in range(N_CORES)
    ]


def _run(x, weight, bias, trace=False):
    res = run_bass_kernel_spmd(
        _get_nc(),
        _in_maps(x, weight, bias),
        core_ids=list(range(N_CORES)),
        trace=trace,
    )
    out = np.concatenate(
        [res.results[i]["out"] for i in range(N_CORES)], axis=0
    ).astype(np.float32)
    return out, res


def kernel(x, weight, bias):
    out, _ = _run(x, weight, bias, trace=False)
    return out


def run_profiled(x, weight, bias):
    out, res = _run(x, weight, bias, trace=True)
    return out, res.exec_time_ns
